# revision 4
# baseline (speedup 1.0000x reference)
"""Trainium2 Bass kernel for nn_Connector — optimized v2.

Data-parallel over 8 NeuronCores; per core 1024 tokens processed as 4
pairs of 128-token tiles.

Key structure per pair (tiles a, b):
  - ms:      ACT Square with fused accum -> ssq per token
  - rsqrt:   Quake-style int hack + 2 Newton iterations on DVE (avoids
             ACT Ln -> kills the per-tile activation-table thrash)
  - G:       PE transposes 128x128 blocks of both tiles into [feat, tokA|tokB]
             then 64 accumulating f32r matmuls vs phi chunks (256 useful
             moving cols each, no junk tail)
  - sinkhorn: 8 linear-space iterations, batched for the pair [128, 2, 4, 4]
  - mixing:  tokens split in 4 groups of 32; stationary W [32, 128] holds
             diag-spread M/H values so each 512-col chunk of the output is
             streamed once per source; groups sit at tile_position (32g, 0)
             so they pack into distinct PE row-strips on hardware
  - out:     PSUM->SBUF copies on ACT, 1 MiB per-group out DMAs (sync queue)
"""
import os
import sys

for _p in (
    "/opt/trn_rl_repo",
    "/opt/trn_rl_repo/pypackages",
    "/root/.axon_site/_ro/trn_rl_repo",
    "/root/.axon_site/_ro/pypackages",
):
    if os.path.isdir(_p) and _p not in sys.path:
        sys.path.append(_p)

from contextlib import ExitStack

import numpy as np

import concourse.bacc as bacc
import concourse.bass as bass
import concourse.tile as tile
from concourse import mybir
from concourse.bass_utils import run_bass_kernel_spmd

F32 = mybir.dt.float32
F32R = mybir.dt.float32r
I32 = mybir.dt.int32
BF16 = mybir.dt.bfloat16
AF = mybir.ActivationFunctionType
ALU = mybir.AluOpType
AX = mybir.AxisListType

B, S, N, C = 4, 2048, 4, 2048
NCORES = 8
TOK = B * S
TPC = TOK // NCORES        # 1024 tokens per core
P = 128
NTILES = TPC // P          # 8 tiles per core
NPAIRS = NTILES // 2
F = N * C                  # 8192
NFB = F // P               # 64 feature blocks
G20 = N + N * N            # 20
EPS = 1e-5
ITERS = 8                  # sinkhorn iters (20 in ref; 10 is within 6e-5)
MAGIC = 0x5F3759DF


def _kernel_body(ctx, tc, out_d, res_d, outp_d, phi_d, bias_d, eye_d, eyem_d):
    nc = tc.nc

    consts = ctx.enter_context(tc.tile_pool(name="consts", bufs=1))
    res_pool = ctx.enter_context(tc.tile_pool(name="res", bufs=3))
    outp_pool = ctx.enter_context(tc.tile_pool(name="outp", bufs=2))
    junk_pool = ctx.enter_context(tc.tile_pool(name="junk", bufs=1))
    tsb_pool = ctx.enter_context(tc.tile_pool(name="tsb", bufs=3))
    pair_pool = ctx.enter_context(tc.tile_pool(name="pair", bufs=2))
    w2_pool = ctx.enter_context(tc.tile_pool(name="w2", bufs=2))
    osb_pool = ctx.enter_context(tc.tile_pool(name="osb", bufs=2))

    tp_psum = ctx.enter_context(tc.tile_pool(name="tp_ps", bufs=2, space="PSUM"))
    g_psum = ctx.enter_context(tc.tile_pool(name="g_ps", bufs=1, space="PSUM"))
    gt_psum = ctx.enter_context(tc.tile_pool(name="gt_ps", bufs=1, space="PSUM"))
    mix_psum = ctx.enter_context(tc.tile_pool(name="mix_ps", bufs=1, space="PSUM"))

    # constants
    phi_sb = consts.tile([P, NFB, G20], F32R)
    nc.sync.dma_start(phi_sb[:], phi_d.rearrange("(c p) m -> p c m", p=P))
    eye_sb = consts.tile([P, P], F32R)
    nc.sync.dma_start(eye_sb[:], eye_d[:])
    eyem_sb = consts.tile([P, 32], F32)
    nc.sync.dma_start(eyem_sb[:], eyem_d[:])
    eye20 = consts.tile([G20, G20], F32)
    nc.vector.tensor_copy(eye20[:], eye_sb[0:G20, 0:G20].bitcast(F32))
    bias_sb = consts.tile([P, G20], F32)
    nc.sync.dma_start(bias_sb[:], bias_d[:].partition_broadcast(P))
    zero_sb = consts.tile([P, 1], F32)
    nc.vector.memset(zero_sb[:], 0.0)

    for p_idx in range(NPAIRS):
        res_t = []
        outp_t = []
        for t in range(2):
            tok = slice((2 * p_idx + t) * P, (2 * p_idx + t + 1) * P)
            rt = res_pool.tile([P, F], F32R)
            nc.sync.dma_start(rt[:], res_d[tok, :])
            res_t.append(rt)
        for t in range(2):
            tok = slice((2 * p_idx + t) * P, (2 * p_idx + t + 1) * P)
            ot = outp_pool.tile([P, C], F32R)
            nc.sync.dma_start(ot[:], outp_d[tok, :])
            outp_t.append(ot)

        # ---- sum of squares per token (ACT Square, fused accumulate) ----
        ssq = pair_pool.tile([P, 2], F32)
        for t in range(2):
            junk = junk_pool.tile([P, F], mybir.dt.float8e4)
            nc.scalar.activation(out=junk[:], in_=res_t[t][:].bitcast(F32),
                                 func=AF.Square, bias=zero_sb[:],
                                 accum_out=ssq[:, t:t + 1])

        # ---- rsq = rsqrt(ssq/F + eps): Quake hack + 2 Newton iters (DVE) --
        v = pair_pool.tile([P, 2], F32)
        nc.vector.tensor_scalar(out=v[:], in0=ssq[:], scalar1=float(1.0 / F),
                                scalar2=EPS, op0=ALU.mult, op1=ALU.add)
        y = pair_pool.tile([P, 2], F32)
        yi = y[:].bitcast(I32)
        nc.vector.tensor_scalar(out=yi, in0=v[:].bitcast(I32), scalar1=1,
                                scalar2=-1, op0=ALU.logical_shift_right,
                                op1=ALU.bitwise_xor)
        nc.vector.tensor_scalar(out=yi, in0=yi, scalar1=MAGIC + 1,
                                scalar2=None, op0=ALU.add)
        tq = pair_pool.tile([P, 2], F32)
        for _ in range(2):
            nc.vector.tensor_tensor(out=tq[:], in0=y[:], in1=y[:], op=ALU.mult)
            nc.vector.tensor_tensor(out=tq[:], in0=tq[:], in1=v[:], op=ALU.mult)
            nc.vector.tensor_scalar(out=tq[:], in0=tq[:], scalar1=-0.5,
                                    scalar2=1.5, op0=ALU.mult, op1=ALU.add)
            nc.vector.tensor_tensor(out=y[:], in0=y[:], in1=tq[:], op=ALU.mult)

        # ---- G = flat @ phi via PE transposes + accumulating matmuls ----
        gcv = g_psum.tile([G20, 2 * P], F32)
        g_ps = gcv[:]
        for fbq in range(NFB // 2):
            t_ps = tp_psum.tile([P, 512], F32R)
            for q in range(4):
                fb, t = 2 * fbq + q // 2, q % 2
                nc.tensor.transpose(t_ps[:, q * P:(q + 1) * P],
                                    res_t[t][:, fb * P:(fb + 1) * P], eye_sb[:])
            t_sb = tsb_pool.tile([P, 512], F32R)
            nc.vector.tensor_copy(t_sb[:], t_ps[:])
            nc.tensor.matmul(g_ps, phi_sb[:, 2 * fbq, :], t_sb[:, 0:256],
                             start=(fbq == 0), stop=False)
            nc.tensor.matmul(g_ps, phi_sb[:, 2 * fbq + 1, :], t_sb[:, 256:512],
                             start=False, stop=(fbq == NFB // 2 - 1))

        g_sb = pair_pool.tile([G20, 2 * P], F32)
        nc.vector.tensor_copy(g_sb[:], g_ps)

        # tilde[:, t, :] = G_t * rsq_t + bias
        tilde = pair_pool.tile([P, 2, G20], F32)
        for t in range(2):
            gtv = gt_psum.tile([P, G20], F32, name="gtv")
            gt_ps = gtv[:]
            nc.tensor.transpose(gt_ps, g_sb[:, t * P:(t + 1) * P], eye20[:])
            nc.vector.tensor_scalar_mul(tilde[:, t, :], in0=gt_ps,
                                        scalar1=y[:, t:t + 1])
            nc.vector.tensor_add(tilde[:, t, :], tilde[:, t, :], bias_sb[:])

        # ---- hv = sigmoid(tilde_post) (the 2x is folded into W build) ----
        hv = pair_pool.tile([P, 2, N], F32)
        nc.scalar.activation(out=hv[:], in_=tilde[:, :, 0:N], func=AF.Exp,
                             scale=-1.0, bias=zero_sb[:])
        nc.vector.tensor_scalar_add(hv[:], in0=hv[:], scalar1=1.0)
        nc.vector.reciprocal(hv[:], hv[:])

        # ---- sinkhorn (linear space), batched over the pair ----
        m_sb = pair_pool.tile([P, 2, N * N], F32)
        nc.scalar.activation(out=m_sb[:], in_=tilde[:, :, N:G20], func=AF.Exp,
                             bias=zero_sb[:])
        m4 = m_sb[:].rearrange("p t (i j) -> p t i j", i=N)
        rs = pair_pool.tile([P, 2, N], F32)
        rr = pair_pool.tile([P, 2, N], F32)
        cs = pair_pool.tile([P, 2, N], F32)
        cr = pair_pool.tile([P, 2, N], F32)
        rr_b = rr[:].unsqueeze(3).broadcast_to([P, 2, N, N])
        cr_b = cr[:].unsqueeze(2).broadcast_to([P, 2, N, N])
        for _ in range(ITERS):
            nc.vector.tensor_reduce(out=rs[:], in_=m4, axis=AX.X, op=ALU.add)
            nc.vector.reciprocal(rr[:], rs[:])
            nc.vector.tensor_tensor(out=m4, in0=m4, in1=rr_b, op=ALU.mult)
            nc.vector.tensor_reduce(out=cs[:], in_=m4.transpose([0, 1, 3, 2]),
                                    axis=AX.X, op=ALU.add)
            nc.vector.reciprocal(cr[:], cs[:])
            nc.vector.tensor_tensor(out=m4, in0=m4, in1=cr_b, op=ALU.mult)

        # ---- mixing ----
        # Stationary W[32tok, 5src, 128] with cols token-major (c = tt*4+i):
        # psum partition p = tt*4+i, so each group's output DMAs out as
        # 32 contiguous token rows x [i, 2048-contiguous cols].
        for t in range(2):
            k = 2 * p_idx + t
            w2 = w2_pool.tile([P, N + 1, P], F32R)
            w2v = w2[:].rearrange("p s (tt i) -> p s i tt", i=N)
            for j in range(N):
                for i in range(N):
                    nc.vector.tensor_scalar_mul(
                        w2v[:, j, i, :], in0=eyem_sb[:],
                        scalar1=m_sb[:, t, i * N + j:i * N + j + 1])
            for i in range(N):
                nc.vector.tensor_scalar(
                    out=w2v[:, N, i, :], in0=eyem_sb[:],
                    scalar1=hv[:, t, i:i + 1], scalar2=2.0,
                    op0=ALU.mult, op1=ALU.mult)

            osb = osb_pool.tile([P, 4, 2048], F32)
            for ch in range(4):
                ps = [mix_psum.tile([P, 512], F32, name=f"mx{g}")
                      for g in range(4)]
                for src in (0, 1, 2, 3, N):
                    for g in range(4):
                        if src < N:
                            rhs = res_t[t][g * 32:(g + 1) * 32,
                                           src * C + ch * 512:src * C + ch * 512 + 512]
                        else:
                            rhs = outp_t[t][g * 32:(g + 1) * 32,
                                            ch * 512:ch * 512 + 512]
                        nc.tensor.matmul(ps[g][:], w2[g * 32:(g + 1) * 32, src, :],
                                         rhs, start=(src == 0), stop=(src == N),
                                         tile_position=(32 * g, 0))
                for g in range(4):
                    nc.scalar.copy(out=osb[:, g, ch * 512:(ch + 1) * 512],
                                   in_=ps[g][:])
            for g in range(4):
                ov = out_d[k * P + 32 * g:k * P + 32 * g + 32, :].rearrange(
                    "tt (i cc) -> tt i cc", i=N)
                nc.sync.dma_start(ov, osb[:, g, :])


def build_nc(reps=1):
    nc = bacc.Bacc("TRN2", target_bir_lowering=False)
    res_d = nc.declare_dram_parameter("residual", [TPC, F], F32R, isOutput=False)
    outp_d = nc.declare_dram_parameter("outp", [TPC, C], F32R, isOutput=False)
    phi_d = nc.declare_dram_parameter("phi", [F, G20], F32R, isOutput=False)
    bias_d = nc.declare_dram_parameter("bias", [G20], F32, isOutput=False)
    eye_d = nc.declare_dram_parameter("eye", [P, P], F32R, isOutput=False)
    eyem_d = nc.declare_dram_parameter("eyemod", [P, 32], F32, isOutput=False)
    out_d = nc.declare_dram_parameter("out", [TPC, F], F32, isOutput=True)
    with tile.TileContext(nc) as tc:
        for _ in range(reps):
            with ExitStack() as ctx:
                _kernel_body(ctx, tc, out_d[:], res_d[:], outp_d[:], phi_d[:],
                             bias_d[:], eye_d[:], eyem_d[:])
    if not nc.is_finalized():
        nc.finalize()
    return nc


_NC_CACHE = {}


def _get_nc():
    if "nc" not in _NC_CACHE:
        _NC_CACHE["nc"] = build_nc()
    return _NC_CACHE["nc"]


def _prep_in_maps(residual, output, rms_scale, phi_post, phi_res, b_post,
                  b_res, alpha_post, alpha_res):
    residual = np.ascontiguousarray(np.asarray(residual, dtype=np.float32))
    output = np.ascontiguousarray(np.asarray(output, dtype=np.float32))
    rms_scale = np.asarray(rms_scale, dtype=np.float32)
    phi_post = np.asarray(phi_post, dtype=np.float32)
    phi_res = np.asarray(phi_res, dtype=np.float32)
    b_post = np.asarray(b_post, dtype=np.float32)
    b_res = np.asarray(b_res, dtype=np.float32)
    a_post = float(np.asarray(alpha_post))
    a_res = float(np.asarray(alpha_res))

    phi_cat = np.ascontiguousarray(
        np.concatenate([a_post * phi_post, a_res * phi_res], axis=1)
        * rms_scale[:, None]).astype(np.float32)
    bias_cat = np.concatenate([b_post, b_res.reshape(-1)]).astype(np.float32)
    eye = np.eye(P, dtype=np.float32)
    eyemod = np.tile(np.eye(32, dtype=np.float32), (4, 1))

    res_flat = residual.reshape(TOK, F)
    outp_flat = output.reshape(TOK, C)
    in_maps = []
    for c in range(NCORES):
        sl = slice(c * TPC, (c + 1) * TPC)
        in_maps.append({
            "residual": np.ascontiguousarray(res_flat[sl]),
            "outp": np.ascontiguousarray(outp_flat[sl]),
            "phi": phi_cat,
            "bias": bias_cat,
            "eye": eye,
            "eyemod": eyemod,
        })
    return in_maps


def run_sharded(trace=False, **inputs):
    """Run on hardware; returns (full_output, exec_time_ns)."""
    in_maps = _prep_in_maps(**inputs)
    nc = _get_nc()
    r = run_bass_kernel_spmd(nc, in_maps, list(range(NCORES)), trace=trace)
    outs = [np.asarray(r.results[c]["out"]) for c in range(NCORES)]
    full = np.concatenate(outs, axis=0).reshape(B, S, N, C).astype(np.float32)
    return full, r.exec_time_ns


def _spot_check(full, residual, output, rms_scale, phi_post, phi_res,
                b_post, b_res, alpha_post, alpha_res,
                tok_idx=(0, 3000, 5531, 8191)):
    """Recompute a few tokens in numpy; guards against transient bad runs."""
    res = np.asarray(residual, np.float64).reshape(TOK, N, C)
    outp = np.asarray(output, np.float64).reshape(TOK, C)
    got = np.asarray(full, np.float32).reshape(TOK, N, C)
    phi_post = np.asarray(phi_post, np.float64)
    phi_res = np.asarray(phi_res, np.float64)
    scale = np.asarray(rms_scale, np.float64)
    b_res = np.asarray(b_res, np.float64)
    b_post = np.asarray(b_post, np.float64)
    worst = 0.0
    for ti in tok_idx:
        flat = res[ti].reshape(F)
        ms = np.mean(flat * flat)
        norm = flat / np.sqrt(ms + EPS) * scale
        tp = float(alpha_post) * (norm @ phi_post) + b_post
        tr = (float(alpha_res) * (norm @ phi_res)).reshape(N, N) + b_res
        H = 2.0 / (1.0 + np.exp(-tp))
        M = np.exp(tr)
        for _ in range(20):
            M = M / M.sum(1, keepdims=True)
            M = M / M.sum(0, keepdims=True)
        exp = M @ res[ti] + H[:, None] * outp[ti][None, :]
        e = np.linalg.norm(got[ti] - exp) / (np.linalg.norm(exp) + 1e-30)
        worst = max(worst, e)
    return worst


def kernel(**inputs):
    full, _ = run_sharded(trace=False, **inputs)
    if _spot_check(full, **inputs) > 1e-2:
        # transient bad execution (seen once on a cold device): retry
        full, _ = run_sharded(trace=False, **inputs)
    return full
